# revision 127
# baseline (speedup 1.0000x reference)
"""NeuroSAT message-passing GNN on 8 TRN2 NeuronCores (Bass/Tile).

Sharding: clause dim sharded 8-way (2048 padded clauses/core); literal dim
permuted so core i owns problem i's 500 vars (+12 pads) as 1024 lit rows
(512 pos + 512 neg).  Per round (pipelined):
  GEMM2 groups 0,1 -> AllToAll half0 ; groups 2,3 -> AllToAll half1 (fp8)
  partials summed locally on DVE (f32), L-LSTM + L_pre MLP per half,
  AllGather halves of L_pre (fp8, Shared-output Mesh)
  GEMM1 LC.T = L_pre.T @ B1 ; C-LSTM ; C_pre MLP ; repeat
M (counts) is exact in fp8e4m3; fp8 M blocks stream from HBM as the moving
operand against fp8 stationary activations (DoubleRow).  AllToAll is used
instead of ReduceScatter because it always runs the O(1)-hop Mesh algorithm
(RS picks RDH at this size: ~2x slower); the 8 partial blocks are reduced
on the vector engine.  Zero-contribution dummy matmuls keep the PE HAM-warm
(K=8/8 clock) across the residual collective waits.
"""

import numpy as np
import ml_dtypes

import concourse.bass as bass
import concourse.bacc as bacc
import concourse.mybir as mybir
import concourse.tile as tile
from concourse import bass_utils

F32 = mybir.dt.float32
BF16 = mybir.dt.bfloat16
FP8 = mybir.dt.float8e4
AF = mybir.ActivationFunctionType

N_CORES = 8
DIM = 128
N_ROUNDS = 16
N_VARS = 4000
VPC = 500            # real vars per core (= vars per problem)
VPAD = 512           # padded vars per core
LL = 2 * VPAD        # 1024 lit rows per core
LPAD = N_CORES * LL  # 8192
CC = 2048            # padded clauses per core
CPAD = N_CORES * CC  # 16384
KL = LPAD // 128     # 64 k-tiles over lits
KC = CC // 128       # 16 k-tiles over clauses

# GEMM2 groups: group g computes 512-lit chunks J_SETS[g]; chunk j covers
# local lit rows [512*(j%2)...) of destination core j//2.  Groups 0,1 cover
# all even j (RS half 0 = every core's rows 0:512); groups 2,3 odd j.
J_SETS = [[0, 2, 4, 6], [8, 10, 12, 14], [1, 3, 5, 7], [9, 11, 13, 15]]

N_WARM1 = 8          # dummy MMs per gate group, L half 0 (A2A_0 wait)
N_WARM_G1 = 12       # dummy MM prefix on GEMM1 (AG + load window)
N_WARM_MID = 0       # dummy MMs between GEMM1 halves (AG_1 tail)
N_WARM2 = 6          # dummy MMs per gate group, L half 1 (A2A_1 wait)
N_B2_STREAM = 7      # streamed b2 blocks 0..6 (g0/g1); 7..15 stay resident.
                     # The streamed blocks belong to GEMM2's FIRST groups so
                     # all HBM streaming finishes before A2A_0 fires; g2/g3
                     # (which run inside the A2A windows) touch only SBUF,
                     # leaving HBM quiet for the collectives.

nbf = ml_dtypes.bfloat16
nf8 = ml_dtypes.float8_e4m3

_CACHE = {}


def _build():
    """Build + compile the SPMD program once (shape-only, no input values)."""
    if "nc" in _CACHE:
        return _CACHE["nc"]

    nc = bacc.Bacc("TRN2", target_bir_lowering=False, debug=False,
                   num_devices=N_CORES)

    def din(name, shape, dt):
        return nc.dram_tensor(name, shape, dt, kind="ExternalInput")

    # b1: 16 packed groups of 4 k-tiles; rows ordered [half h, core c, r<512]
    b1 = din("b1", [KL // 4, DIM, 4 * CC], FP8)
    # b2[g]: group g's 16 k-tiles packed 4-per-DMA: [4 groups, 4 qgrp, 128, 4*2048]
    b2 = din("b2", [4, 4, DIM, 4 * 2048], FP8)
    lh0t = din("lh0t", [DIM, LL], BF16)
    ch0t = din("ch0t", [DIM, CC], BF16)
    id128 = din("id128", [DIM, DIM], BF16)
    # folded layer-3 biases: LC += deg(clause)*Lmsg_b3, CL += deg(lit)*Cmsg_b3
    lcbias_d = din("lcbias", [DIM, CC], BF16)
    clbias_d = din("clbias", [DIM, LL], BF16)

    w = {}
    for p in ("lmsg", "cmsg", "lvote"):
        for i in (1, 2, 3):
            shp = [DIM, 1] if (p == "lvote" and i == 3) else [DIM, DIM]
            w[f"{p}_w{i}t"] = din(f"{p}_w{i}t", shp, BF16)
            bshp = [1, 1] if (p == "lvote" and i == 3) else [DIM, 1]
            w[f"{p}_b{i}"] = din(f"{p}_b{i}", bshp, F32)
    w["cu_wiht"] = din("cu_wiht", [DIM, 4 * DIM], BF16)
    w["cu_whht"] = din("cu_whht", [DIM, 4 * DIM], BF16)
    w["lu_wiht_cl"] = din("lu_wiht_cl", [DIM, 4 * DIM], BF16)
    w["lu_wiht_fl"] = din("lu_wiht_fl", [DIM, 4 * DIM], BF16)
    w["lu_whht"] = din("lu_whht", [DIM, 4 * DIM], BF16)
    cu_bias_d = din("cu_bias", [4, DIM], F32)
    lu_bias_d = din("lu_bias", [4, DIM], F32)

    vote_out = nc.dram_tensor("vote", [1, LL], F32, kind="ExternalOutput")

    with tile.TileContext(nc) as tc, \
         tc.tile_pool(name="const", bufs=1) as const, \
         tc.tile_pool(name="sb", bufs=2) as sb, \
         tc.tile_pool(name="sb3", bufs=2) as sb3, \
         tc.tile_pool(name="ps", bufs=6, space="PSUM") as ps, \
         tc.tile_pool(name="pstr", bufs=2, space="PSUM") as pstr, \
         tc.tile_pool(name="dram", bufs=2, space="DRAM") as dram:

        # ---- resident b2: blocks N_B2_STREAM..15 of [128, 8192] fp8
        b2r = const.tile([DIM, (16 - N_B2_STREAM) * 4 * 2048], FP8, tag="b2r")
        for g in range(4):
            for q in range(4):
                blk = g * 4 + q
                if blk < N_B2_STREAM:
                    continue
                sl = slice((blk - N_B2_STREAM) * 8192,
                           (blk - N_B2_STREAM + 1) * 8192)
                nc.sync.dma_start(b2r[:, sl], b2.ap()[g, q, :, :])

        # ---- load constants/weights into SBUF
        cw = {}
        for k in w:
            t = const.tile(list(w[k].shape), w[k].dtype, tag=f"cw_{k}")
            nc.sync.dma_start(t[:], w[k].ap())
            cw[k] = t
        for k, dte in (("cu_bias", cu_bias_d), ("lu_bias", lu_bias_d)):
            t = const.tile([DIM, 4], F32, tag=f"cw_{k}")
            nc.sync.dma_start(t[:], dte.ap().rearrange("g p -> p g"))
            cw[k] = t
        idt = const.tile([DIM, DIM], BF16, tag="idt")
        nc.sync.dma_start(idt[:], id128.ap())
        zbf = const.tile([DIM, 512], BF16, tag="zbf")
        nc.vector.memset(zbf[:], 0.0)
        lcbias = const.tile([DIM, CC], BF16, tag="lcbias")
        nc.sync.dma_start(lcbias[:], lcbias_d.ap())
        clbias = const.tile([DIM, LL], BF16, tag="clbias")
        nc.sync.dma_start(clbias[:], clbias_d.ap())

        # ---- persistent state (feature-major)
        lht = const.tile([DIM, LL], BF16, tag="lht")
        lct = const.tile([DIM, LL], BF16, tag="lct")
        cht = const.tile([DIM, CC], BF16, tag="cht")
        cct = const.tile([DIM, CC], BF16, tag="cct")
        nc.sync.dma_start(lht[:], lh0t.ap())
        nc.sync.dma_start(cht[:], ch0t.ap())
        nc.vector.memset(lct[:], 0.0)
        nc.vector.memset(cct[:], 0.0)

        def dma2(dst, src):
            """Split a [128, N] transfer across two DMA queues by partitions."""
            nc.sync.dma_start(dst[0:64, :], src[0:64, :])
            nc.sync.dma_start(dst[64:DIM, :], src[64:DIM, :])

        def mlp_chunk(x, pfx, sl, n, out_dt=BF16, tagsfx="", layers=(1, 2, 3)):
            """MLP layers on columns sl (chunks of <=512) of x [128, *]."""
            cur = x
            for li in layers:
                wt = cw[f"{pfx}_w{li}t"]
                bt = cw[f"{pfx}_b{li}"]
                m = wt.shape[1]
                o = sb.tile([m, n], out_dt if li == 3 else BF16, bufs=1,
                            tag=f"{pfx}_h{li}{tagsfx}", name=f"{pfx}_h{li}{tagsfx}")
                for rc in range(n // 512):
                    c0 = rc * 512
                    pt = ps.tile([m, 512], F32, tag="ps", name="mlp_ps")
                    src = cur[:, sl.start + c0:sl.start + c0 + 512] if li == layers[0] \
                        else cur[:, c0:c0 + 512]
                    nc.tensor.matmul(pt[:], wt[:], src, start=True, stop=True)
                    func = AF.Relu if li < 3 else AF.Identity
                    nc.scalar.activation(o[:, c0:c0 + 512], pt[:], func,
                                         bias=bt[:, 0:1])
                cur = o
            return cur

        def layer3_T(h2, pfx, n_tiles, dst, dst_off):
            """Transposed MLP layer 3: k-tile t of dst gets (h2_t)^T @ W3^T,
            i.e. L_pre^T/C_pre^T [128 rows, 128 feat] directly -- no PE
            transposes.  Layer-3 bias is folded into lcbias/clbias.  Psums
            are batched 4 k-tiles per [128,512] ring tile."""
            w3t = cw[f"{pfx}_w3t"]
            for t in range(n_tiles):
                pt = pstr.tile([DIM, DIM], F32, tag="pstr", name=f"{pfx}_l3t")
                nc.tensor.matmul(pt[:], h2[:, t * DIM:(t + 1) * DIM], w3t[:],
                                 start=True, stop=True)
                osl = slice((dst_off + t) * DIM, (dst_off + t + 1) * DIM)
                nc.vector.tensor_copy(dst[:, osl], pt[:])

        def lstm_elementwise(gps, bias, c_st, h_st, rc0, n):
            """gps: 4 psum tiles [128, n] (i,f,g,o); updates states [:, rc0:rc0+n]."""
            sl = slice(rc0, rc0 + n)
            sig_i = sb.tile([DIM, n], BF16, tag="lw_si", bufs=1, name="sig_i")
            sig_f = sb.tile([DIM, n], BF16, tag="lw_sf", bufs=1, name="sig_f")
            tng = sb.tile([DIM, n], BF16, tag="lw_tg", bufs=1, name="tng")
            sig_o = sb.tile([DIM, n], BF16, tag="lw_so", bufs=1, name="sig_o")
            nc.scalar.activation(sig_i[:], gps[0][:], AF.Sigmoid, bias=bias[:, 0:1])
            nc.scalar.activation(sig_f[:], gps[1][:], AF.Sigmoid, bias=bias[:, 1:2])
            nc.scalar.activation(tng[:], gps[2][:], AF.Tanh, bias=bias[:, 2:3])
            nc.scalar.activation(sig_o[:], gps[3][:], AF.Sigmoid, bias=bias[:, 3:4])
            t1 = sb.tile([DIM, n], BF16, tag="lw_t1", bufs=1, name="t1")
            nc.vector.tensor_mul(t1[:], sig_f[:], c_st[:, sl])
            t2 = sb.tile([DIM, n], BF16, tag="lw_t2", bufs=1, name="t2")
            nc.vector.tensor_mul(t2[:], sig_i[:], tng[:])
            nc.vector.tensor_add(c_st[:, sl], t1[:], t2[:])
            tnc = sb.tile([DIM, n], BF16, tag="lw_tc", bufs=1, name="tnc")
            nc.scalar.activation(tnc[:], c_st[:, sl], AF.Tanh)
            nc.vector.tensor_mul(h_st[:, sl], sig_o[:], tnc[:])

        def c_lstm(lct_ps):
            """C-LSTM over 4 clause chunks."""
            for rc in range(4):
                sl = slice(rc * 512, (rc + 1) * 512)
                lc_sb = sb.tile([DIM, 512], BF16, tag="lc_sb", bufs=2,
                                name=f"lc_sb{rc}")
                nc.vector.tensor_add(lc_sb[:], lct_ps[rc][:], lcbias[:, sl])
                gps = [ps.tile([DIM, 512], F32, tag="ps", name=f"cg{i}")
                       for i in range(4)]
                for g in range(4):
                    gsl = slice(g * DIM, (g + 1) * DIM)
                    nc.tensor.matmul(gps[g][:], cw["cu_wiht"][:, gsl],
                                     lc_sb[:], start=True, stop=False)
                    nc.tensor.matmul(gps[g][:], cw["cu_whht"][:, gsl],
                                     cht[:, sl], start=False, stop=True)
                lstm_elementwise(gps, cw["cu_bias"], cct, cht, rc * 512, 512)

        def c_mlp_half(ch, cpre_kt):
            """C_pre MLP (transposed layer 3) for clause half ch."""
            h2 = mlp_chunk(cht, "cmsg", slice(ch * 1024, (ch + 1) * 1024),
                           1024, layers=(1, 2))
            layer3_T(h2, "cmsg", 8, cpre_kt, ch * 8)

        N_B2_PRE = 3     # streamed-b2 window depth

        def b2_fetch(i):
            t = sb3.tile([DIM, 4 * 2048], FP8, tag="b2t", bufs=N_B2_PRE,
                         name=f"b2t{i}")
            nc.scalar.dma_start(t[:], b2.ap()[i // 4, i % 4, :, :])
            return t

        def gemm2_qrange(cpre_kt, g, cl_ps, b2s, qa, qb):
            """Accumulate GEMM2 group g over q-blocks [qa, qb)."""
            for q in range(qa, qb):
                blk = g * 4 + q
                if blk < N_B2_STREAM:
                    b2t = b2s[blk]
                    b2v = b2t[:].rearrange("p (t c) -> p t c", c=2048)
                else:
                    gsl = slice((blk - N_B2_STREAM) * 8192,
                                (blk - N_B2_STREAM + 1) * 8192)
                    b2v = b2r[:, gsl].rearrange("p (t c) -> p t c", c=2048)
                for kk in (0, 2):
                    k = 4 * q + kk
                    ck = cpre_kt[:, k * DIM:(k + 2) * DIM].rearrange(
                        "p (j d) -> p j d", j=2)
                    for i in range(4):
                        nc.tensor.matmul(
                            cl_ps[i][:], ck,
                            b2v[:, kk:kk + 2, i * 512:(i + 1) * 512],
                            start=(k == 0), stop=(k == KC - 2),
                            perf_mode=mybir.MatmulPerfMode.DoubleRow)
                if blk + N_B2_PRE < N_B2_STREAM:
                    b2s.append(b2_fetch(blk + N_B2_PRE))

        def gemm2_stage(g, cl_ps, rs_bufs):
            """Stage group g's 4 blocks with one contiguous DMA.  Groups 0/1
            copy on the (idle) vector engine so A2A_0 triggers sooner; groups
            2/3 stay on scalar to keep vector free for the A2A_0 reduce that
            runs concurrently."""
            h, b0 = (0, 0) if g == 0 else (0, 4) if g == 1 else \
                    (1, 0) if g == 2 else (1, 4)
            cs4 = sb.tile([DIM, 4 * 512], FP8, tag="cl_st", bufs=1,
                          name=f"cl_st{g}")
            for i in range(4):
                seg = cs4[:, i * 512:(i + 1) * 512]
                if g < 2:
                    nc.vector.tensor_copy(seg, cl_ps[i][:])
                else:
                    nc.scalar.activation(seg, cl_ps[i][:], AF.Identity)
            dst = rs_bufs[h][b0 * DIM:(b0 + 4) * DIM, :].rearrange(
                "(b p) c -> p b c", p=DIM)
            src = cs4[:].rearrange("p (b c) -> p b c", c=512)
            nc.sync.dma_start(dst, src)

        def gemm2_psum(g):
            return [ps.tile([DIM, 512], F32, tag="ps", name=f"cl{g}_{i}")
                    for i in range(4)]

        def gemm2_group(cpre_kt, g, rs_bufs, r, b2s):
            """One GEMM2 group: 4 psum accums over KC k-tiles; stage to buf."""
            cl_ps = gemm2_psum(g)
            gemm2_qrange(cpre_kt, g, cl_ps, b2s, 0, 4)
            gemm2_stage(g, cl_ps, rs_bufs)

        def l_half(h, clt_h, lh_flip, r, n_warm, ag_in):
            """L-LSTM + L_pre MLP + transposes for local half h; returns ag_in."""
            sl = slice(h * 512, (h + 1) * 512)
            fsl = slice((1 - h) * 512, (2 - h) * 512)
            gps = [ps.tile([DIM, 512], F32, tag="ps", name=f"lg{h}_{i}")
                   for i in range(4)]
            # flip/hidden gate matmuls first: they only need lh_flip, so the
            # PE computes them while the A2A exchange is still in flight; the
            # clt matmuls (which wait on the reduce) come last.
            for g in range(4):
                gsl = slice(g * DIM, (g + 1) * DIM)
                for wi in range(n_warm):
                    nc.tensor.matmul(gps[g][:], idt[:], zbf[:],
                                     start=(wi == 0), stop=False)
                nc.tensor.matmul(gps[g][:], cw["lu_wiht_fl"][:, gsl],
                                 lh_flip[:, fsl], start=(n_warm == 0),
                                 stop=False)
                nc.tensor.matmul(gps[g][:], cw["lu_whht"][:, gsl],
                                 lh_flip[:, sl], start=False, stop=False)
            for g in range(4):
                gsl = slice(g * DIM, (g + 1) * DIM)
                for b in range(4):
                    nc.tensor.matmul(gps[g][:], cw["lu_wiht_cl"][:, gsl],
                                     clt_h[:, b * 512:(b + 1) * 512],
                                     start=False, stop=(b == 3))
            lstm_elementwise(gps, cw["lu_bias"], lct, lht, h * 512, 512)
            stage_lpre(h, ag_in)

        def stage_lpre(h, ag_in):
            """L_pre^T k-tiles for local half h -> ag_in rows [h*512:(h+1)*512]."""
            h2 = mlp_chunk(lht, "lmsg", slice(h * 512, (h + 1) * 512), 512,
                           tagsfx=f"_{h}", layers=(1, 2))
            lpt = sb.tile([DIM, 4 * DIM], FP8, tag=f"lpt_{h}", bufs=1,
                          name=f"lpt_{h}")
            layer3_T(h2, "lmsg", 4, lpt, 0)
            dst = ag_in[:].rearrange("(t p) d -> p t d", p=DIM)
            nc.sync.dma_start(dst, lpt[:].rearrange("p (t d) -> p t d", d=DIM))

        N_B1_PRE = 5     # b1 window depth (tile bufs / prologue prefetch)

        def b1_fetch(grp):
            t = sb3.tile([DIM, 4 * CC], FP8, tag="b1t", bufs=N_B1_PRE,
                         name=f"b1t{grp}")
            # half-split: the group's kk=0 matmuls (k-tiles 0-1) depend only
            # on the first half, so they start ~1.4us earlier under pacing
            nc.scalar.dma_start(t[:, 0:2 * CC], b1.ap()[grp, :, 0:2 * CC])
            nc.scalar.dma_start(t[:, 2 * CC:4 * CC], b1.ap()[grp, :, 2 * CC:])
            return t

        def gemm1_prologue():
            """Prefetch the first b1 groups; fires as the prior GEMM1 ends."""
            return [b1_fetch(j) for j in range(N_B1_PRE)]

        def gemm1(lpre_sb, n_warm, pre):
            """GEMM1: LC.T [128, 2048] psum accums over 64 packed k-tiles.

            Each group's refill DMA is issued right after the matmuls that
            free its buffer slot, so the scalar HWDGE ring never stalls."""
            tiles = list(pre)
            lct_ps = [ps.tile([DIM, 512], F32, tag="ps", name=f"g1_{i}")
                      for i in range(4)]
            for wi in range(n_warm):
                nc.tensor.matmul(lct_ps[wi % 4][:], idt[:], zbf[:],
                                 start=(wi < 4), stop=False)
            for grp in range(KL // 4):
                if grp == 8:
                    # bridge the AG half-1 wait without letting HAM cool
                    for wi in range(N_WARM_MID):
                        nc.tensor.matmul(lct_ps[wi % 4][:], idt[:], zbf[:],
                                         start=False, stop=False)
                b1t = tiles[grp]
                b1v = b1t[:].rearrange("p (t c) -> p t c", c=CC)
                for kk in (0, 2):
                    k = 4 * grp + kk
                    lf = lpre_sb[grp]
                    lk = lf[:, kk * DIM:(kk + 2) * DIM].rearrange(
                        "p (j d) -> p j d", j=2)
                    for c4 in range(4):
                        nc.tensor.matmul(
                            lct_ps[c4][:], lk,
                            b1v[:, kk:kk + 2, c4 * 512:(c4 + 1) * 512],
                            start=(k == 0 and n_warm == 0),
                            stop=(k == KL - 2),
                            perf_mode=mybir.MatmulPerfMode.DoubleRow)
                if grp + N_B1_PRE < KL // 4:
                    tiles.append(b1_fetch(grp + N_B1_PRE))
            return lct_ps

        def load_lpre(ag_outs):
            """Load AG halves as 16 per-group tiles of 4 k-tiles each.

            b1 groups are half-major [half, core, r]: group g = (half g//8,
            core g%8), so groups 0-7 depend only on AG half 0."""
            lpre_sb = []
            for g in range(16):
                h, c = g // 8, g % 8
                lt = sb.tile([DIM, 4 * DIM], FP8, tag="lpf", bufs=6,
                             name=f"lpf{g}")
                src = ag_outs[h][c * 512:(c + 1) * 512, :]
                s3 = src.rearrange("(t p) d -> p t d", p=DIM)
                d3 = lt[:].rearrange("p (t d) -> p t d", d=DIM)
                nc.sync.dma_start(d3, s3)
                lpre_sb.append(lt)
            return lpre_sb

        rg = [list(range(N_CORES))]

        def collective(kind, op, cin, cout):
            nc.gpsimd.collective_compute(kind, op, replica_groups=rg,
                                         ins=[cin.opt()], outs=[cout.opt()])

        # ====== round 0 head: L_pre^T from Lh0 -> ag_in halves ======
        ag_ins = []
        for h in range(2):
            ag_in = dram.tile([512, DIM], FP8, tag=f"ag_in{h}",
                              name=f"ag_in{h}_init")
            stage_lpre(h, ag_in)
            ag_ins.append(ag_in)

        def a2a_load(h, ro, r):
            """Load A2A output (8 partial blocks) into SBUF.

            Issued for BOTH halves before l_half(0), so half 1's load fires
            the moment A2A_1 completes instead of queuing on the sync ring
            behind lhalf0's AG staging DMA.  Two half-loads per buffer let
            the first pair-sums start while blocks 4-7 are in flight."""
            a2a_sb = sb.tile([DIM, 8 * 512], FP8, tag="a2a_sb", bufs=1,
                             name=f"a2a_sb{h}_{r}")
            src3 = ro[:].rearrange("(b p) c -> p b c", p=DIM)
            dst3 = a2a_sb[:].rearrange("p (b c) -> p b c", c=512)
            nc.sync.dma_start(dst3[:, 0:4], src3[:, 0:4])
            nc.sync.dma_start(dst3[:, 4:8], src3[:, 4:8])
            return a2a_sb

        def a2a_sum(h, a2a_sb, r):
            """Pair-sum the 8 partials -> 4 bf16 partials (exact in bf16).

            The remaining two reduce levels happen for free inside the
            L-LSTM input matmuls (psum-accumulated), on the idle PE."""
            blk = lambda b: a2a_sb[:, b * 512:(b + 1) * 512]
            s1 = sb.tile([DIM, 4 * 512], BF16, tag="a2a_s1", bufs=1,
                         name=f"s1_{h}_{r}")
            for b in range(4):
                nc.vector.tensor_add(s1[:, b * 512:(b + 1) * 512],
                                     blk(2 * b), blk(2 * b + 1))
            # fold deg(lit)*Cmsg_b3 into branch 0 (off the reduce critical path)
            nc.vector.tensor_add(s1[:, 0:512], s1[:, 0:512],
                                 clbias[:, h * 512:(h + 1) * 512])
            return s1

        b1pre = gemm1_prologue()
        for r in range(N_ROUNDS):
            ag_outs = []
            for h in range(2):
                ag_out = dram.tile([4096, DIM], FP8, tag=f"ag_out{h}",
                                   addr_space="Shared", name=f"ag_out{h}_{r}")
                collective("AllGather", mybir.AluOpType.bypass,
                           ag_ins[h], ag_out)
                ag_outs.append(ag_out)
            lpre_sb = load_lpre(ag_outs)
            lct_ps = gemm1(lpre_sb, N_WARM_G1, b1pre)
            if r + 1 < N_ROUNDS:
                b1pre = gemm1_prologue()
            b2s = [b2_fetch(i) for i in range(min(N_B2_STREAM, N_B2_PRE))]

            c_lstm(lct_ps)
            cpre_kt = sb.tile([DIM, KC * DIM], FP8, tag="cpre_kt", bufs=1,
                              name="cpre_kt")
            c_mlp_half(0, cpre_kt)
            c_mlp_half(1, cpre_kt)

            lh_flip = sb.tile([DIM, LL], BF16, tag="lh_flip", bufs=1,
                              name="lh_flip")
            nc.scalar.activation(lh_flip[:], lht[:], AF.Identity)

            rs_bufs = [dram.tile([N_CORES * DIM, 512], FP8, tag=f"rs_in{h}",
                                 name=f"rs_in{h}_{r}") for h in range(2)]
            gemm2_group(cpre_kt, 0, rs_bufs, r, b2s)
            gemm2_group(cpre_kt, 1, rs_bufs, r, b2s)
            ro0 = dram.tile([N_CORES * DIM, 512], FP8, tag="rs_out0",
                            name=f"rs_out0_{r}")
            collective("AllToAll", mybir.AluOpType.bypass, rs_bufs[0], ro0)
            gemm2_group(cpre_kt, 2, rs_bufs, r, b2s)
            gemm2_group(cpre_kt, 3, rs_bufs, r, b2s)

            ro1 = dram.tile([N_CORES * DIM, 512], FP8, tag="rs_out1",
                            name=f"rs_out1_{r}")
            collective("AllToAll", mybir.AluOpType.bypass, rs_bufs[1], ro1)

            ag_ins = [dram.tile([512, DIM], FP8, tag=f"ag_in{h}",
                                name=f"ag_in{h}_{r}") for h in range(2)]
            sb0 = a2a_load(0, ro0, r)
            sb1 = a2a_load(1, ro1, r)
            clt0 = a2a_sum(0, sb0, r)
            l_half(0, clt0, lh_flip, r, N_WARM1, ag_ins[0])

            clt1 = a2a_sum(1, sb1, r)
            l_half(1, clt1, lh_flip, r, N_WARM2, ag_ins[1])

        # ---- vote MLP on final Lh -> [1, 1024] f32
        vt0 = mlp_chunk(lht, "lvote", slice(0, 512), 512, out_dt=F32,
                        tagsfx="_0")
        vt1 = mlp_chunk(lht, "lvote", slice(512, 1024), 512, out_dt=F32,
                        tagsfx="_1")
        nc.sync.dma_start(vote_out.ap()[:, 0:512], vt0[:])
        nc.sync.dma_start(vote_out.ap()[:, 512:1024], vt1[:])

    nc.compile()
    _CACHE["nc"] = nc
    return nc


def _perm_rows(lits):
    """Map global lit index -> permuted row (core-major, 1024 rows/core)."""
    lits = np.asarray(lits)
    neg = lits >= N_VARS
    v = np.where(neg, lits - N_VARS, lits)
    core = v // VPC
    r = v % VPC
    return core * LL + np.where(neg, VPAD + r, r)


def _b1_row_order():
    """B1 rows: [half h, core c, r] -> permuted row c*1024 + h*512 + r."""
    order = np.empty(LPAD, np.int64)
    n = 0
    for h in range(2):
        for c in range(N_CORES):
            order[n:n + 512] = c * LL + h * 512 + np.arange(512)
            n += 512
    return order


def host_prep(inp):
    f32 = np.float32
    idx = inp["L_unpack_indices"].astype(np.int64)
    rows = _perm_rows(idx[:, 0])
    M = np.zeros((LPAD, CPAD), np.float32)
    np.add.at(M, (rows, idx[:, 1]), 1.0)

    # degree vectors for the folded layer-3 biases
    deg_c = M.sum(axis=0)                        # [CPAD] clause degrees
    deg_l = M.sum(axis=1)                        # [LPAD] permuted lit degrees
    lmsg_b3 = inp["Lmsg_b3"].astype(f32)
    cmsg_b3 = inp["Cmsg_b3"].astype(f32)

    row_order = _b1_row_order()
    b1s, b2s, lcbs, clbs = [], [], [], []
    for i in range(N_CORES):
        blk = M[:, i * CC:(i + 1) * CC]          # [8192, 2048] permuted rows
        b1o = blk[row_order]                      # AG-concat row order
        # pack 4 k-tiles per DMA group: [16, 128, 4*2048]
        b1p = b1o.reshape(16, 4, DIM, CC).transpose(0, 2, 1, 3) \
                 .reshape(16, DIM, 4 * CC)
        b1s.append(np.ascontiguousarray(b1p).astype(nf8))
        bT = blk.T                                # [2048 clauses, 8192 lits]
        grp = []
        for g in range(4):
            cols = np.concatenate([np.arange(j * 512, (j + 1) * 512)
                                   for j in J_SETS[g]])
            gb = bT[:, cols]                      # [2048, 2048]
            gp = gb.reshape(4, 4, DIM, 2048).transpose(0, 2, 1, 3) \
                   .reshape(4, DIM, 4 * 2048)
            grp.append(gp)
        b2s.append(np.ascontiguousarray(np.stack(grp)).astype(nf8))
        lcbs.append(np.outer(lmsg_b3, deg_c[i * CC:(i + 1) * CC]))
        clbs.append(np.outer(cmsg_b3, deg_l[i * LL:(i + 1) * LL]))

    def bf(x):
        return np.ascontiguousarray(x).astype(nbf)

    l0 = (inp["L_init_w"][:, 0] + inp["L_init_b"]).astype(f32)
    c0 = (inp["C_init_w"][:, 0] + inp["C_init_b"]).astype(f32)
    common = {
        "lh0t": bf(np.repeat(l0[:, None], LL, axis=1)),
        "ch0t": bf(np.repeat(c0[:, None], CC, axis=1)),
        "id128": bf(np.eye(DIM, dtype=f32)),
        "cu_wiht": bf(inp["Cu_wih"].T), "cu_whht": bf(inp["Cu_whh"].T),
        "lu_wiht_cl": bf(inp["Lu_wih"].T[:DIM]),
        "lu_wiht_fl": bf(inp["Lu_wih"].T[DIM:]),
        "lu_whht": bf(inp["Lu_whh"].T),
        "cu_bias": (inp["Cu_bih"] + inp["Cu_bhh"]).astype(f32).reshape(4, DIM),
        "lu_bias": (inp["Lu_bih"] + inp["Lu_bhh"]).astype(f32).reshape(4, DIM),
    }
    for p, P in (("lmsg", "Lmsg"), ("cmsg", "Cmsg"), ("lvote", "Lvote")):
        for i in (1, 2, 3):
            common[f"{p}_w{i}t"] = bf(inp[f"{P}_w{i}"].T)
            bshape = (1, 1) if (p == "lvote" and i == 3) else (DIM, 1)
            common[f"{p}_b{i}"] = inp[f"{P}_b{i}"].astype(f32).reshape(bshape)
    return [dict(common, b1=b1s[i], b2=b2s[i], lcbias=bf(lcbs[i]),
                 clbias=bf(clbs[i])) for i in range(N_CORES)]


def kernel(**inputs):
    inp = {k: np.asarray(v) for k, v in inputs.items()}
    in_maps = host_prep(inp)
    nc = _build()
    res = bass_utils.run_bass_kernel_spmd(nc, in_maps,
                                          core_ids=list(range(N_CORES)))
    probs = np.zeros(N_CORES, np.float32)
    for i in range(N_CORES):
        v = res.results[i]["vote"][0]            # [1024]
        s = v[:VPC].astype(np.float64).sum() + \
            v[VPAD:VPAD + VPC].astype(np.float64).sum()
        probs[i] = np.float32(s / (2 * VPC))
    return probs



# revision 129
# speedup vs baseline: 1.0400x; 1.0400x over previous
"""NeuroSAT message-passing GNN on 8 TRN2 NeuronCores (Bass/Tile).

Sharding: clause dim sharded 8-way (2048 padded clauses/core); literal dim
permuted so core i owns problem i's 500 vars (+12 pads) as 1024 lit rows
(512 pos + 512 neg).  Per round (pipelined):
  GEMM2 groups 0,1 -> AllToAll half0 ; groups 2,3 -> AllToAll half1 (fp8)
  partials summed locally on DVE (f32), L-LSTM + L_pre MLP per half,
  AllGather halves of L_pre (fp8, Shared-output Mesh)
  GEMM1 LC.T = L_pre.T @ B1 ; C-LSTM ; C_pre MLP ; repeat
M (counts) is exact in fp8e4m3; fp8 M blocks stream from HBM as the moving
operand against fp8 stationary activations (DoubleRow).  AllToAll is used
instead of ReduceScatter because it always runs the O(1)-hop Mesh algorithm
(RS picks RDH at this size: ~2x slower); the 8 partial blocks are reduced
on the vector engine.  Zero-contribution dummy matmuls keep the PE HAM-warm
(K=8/8 clock) across the residual collective waits.
"""

import numpy as np
import ml_dtypes

import concourse.bass as bass
import concourse.bacc as bacc
import concourse.mybir as mybir
import concourse.tile as tile
from concourse import bass_utils

F32 = mybir.dt.float32
BF16 = mybir.dt.bfloat16
FP8 = mybir.dt.float8e4
AF = mybir.ActivationFunctionType

N_CORES = 8
DIM = 128
N_ROUNDS = 16
N_VARS = 4000
VPC = 500            # real vars per core (= vars per problem)
VPAD = 512           # padded vars per core
LL = 2 * VPAD        # 1024 lit rows per core
LPAD = N_CORES * LL  # 8192
CC = 2048            # padded clauses per core
CPAD = N_CORES * CC  # 16384
KL = LPAD // 128     # 64 k-tiles over lits
KC = CC // 128       # 16 k-tiles over clauses

# GEMM2 groups: group g computes 512-lit chunks J_SETS[g]; chunk j covers
# local lit rows [512*(j%2)...) of destination core j//2.  Groups 0,1 cover
# all even j (RS half 0 = every core's rows 0:512); groups 2,3 odd j.
J_SETS = [[0, 2, 4, 6], [8, 10, 12, 14], [1, 3, 5, 7], [9, 11, 13, 15]]

N_WARM1 = 8          # dummy MMs per gate group, L half 0 (A2A_0 wait)
N_WARM_G1 = 12       # dummy MM prefix on GEMM1 (AG + load window)
N_WARM_MID = 0       # dummy MMs between GEMM1 halves (AG_1 tail)
N_WARM2 = 6          # dummy MMs per gate group, L half 1 (A2A_1 wait)
N_B2_STREAM = 7      # streamed b2 blocks 0..6 (g0/g1); 7..15 stay resident.
                     # The streamed blocks belong to GEMM2's FIRST groups so
                     # all HBM streaming finishes before A2A_0 fires; g2/g3
                     # (which run inside the A2A windows) touch only SBUF,
                     # leaving HBM quiet for the collectives.

nbf = ml_dtypes.bfloat16
nf8 = ml_dtypes.float8_e4m3

_CACHE = {}


def _build():
    """Build + compile the SPMD program once (shape-only, no input values)."""
    if "nc" in _CACHE:
        return _CACHE["nc"]

    nc = bacc.Bacc("TRN2", target_bir_lowering=False, debug=False,
                   num_devices=N_CORES)

    def din(name, shape, dt):
        return nc.dram_tensor(name, shape, dt, kind="ExternalInput")

    # b1: 16 packed groups of 4 k-tiles; rows ordered [half h, core c, r<512]
    b1 = din("b1", [KL // 4, DIM, 4 * CC], FP8)
    # b2[g]: group g's 16 k-tiles packed 4-per-DMA: [4 groups, 4 qgrp, 128, 4*2048]
    b2 = din("b2", [4, 4, DIM, 4 * 2048], FP8)
    lh0t = din("lh0t", [DIM, LL], BF16)
    ch0t = din("ch0t", [DIM, CC], BF16)
    id128 = din("id128", [DIM, DIM], BF16)
    # folded layer-3 biases: LC += deg(clause)*Lmsg_b3, CL += deg(lit)*Cmsg_b3
    lcbias_d = din("lcbias", [DIM, CC], BF16)
    clbias_d = din("clbias", [DIM, LL], BF16)

    w = {}
    for p in ("lmsg", "cmsg", "lvote"):
        for i in (1, 2, 3):
            shp = [DIM, 1] if (p == "lvote" and i == 3) else [DIM, DIM]
            w[f"{p}_w{i}t"] = din(f"{p}_w{i}t", shp, BF16)
            bshp = [1, 1] if (p == "lvote" and i == 3) else [DIM, 1]
            w[f"{p}_b{i}"] = din(f"{p}_b{i}", bshp, F32)
    w["cu_wiht"] = din("cu_wiht", [DIM, 4 * DIM], BF16)
    w["cu_whht"] = din("cu_whht", [DIM, 4 * DIM], BF16)
    w["lu_wiht_cl"] = din("lu_wiht_cl", [DIM, 4 * DIM], BF16)
    w["lu_wiht_fl"] = din("lu_wiht_fl", [DIM, 4 * DIM], BF16)
    w["lu_whht"] = din("lu_whht", [DIM, 4 * DIM], BF16)
    cu_bias_d = din("cu_bias", [4, DIM], F32)
    lu_bias_d = din("lu_bias", [4, DIM], F32)

    vote_out = nc.dram_tensor("vote", [1, LL], F32, kind="ExternalOutput")

    with tile.TileContext(nc) as tc, \
         tc.tile_pool(name="const", bufs=1) as const, \
         tc.tile_pool(name="sb", bufs=2) as sb, \
         tc.tile_pool(name="sb3", bufs=2) as sb3, \
         tc.tile_pool(name="ps", bufs=6, space="PSUM") as ps, \
         tc.tile_pool(name="pstr", bufs=2, space="PSUM") as pstr, \
         tc.tile_pool(name="dram", bufs=2, space="DRAM") as dram:

        # ---- resident b2: blocks N_B2_STREAM..15 of [128, 8192] fp8
        b2r = const.tile([DIM, (16 - N_B2_STREAM) * 4 * 2048], FP8, tag="b2r")
        for g in range(4):
            for q in range(4):
                blk = g * 4 + q
                if blk < N_B2_STREAM:
                    continue
                sl = slice((blk - N_B2_STREAM) * 8192,
                           (blk - N_B2_STREAM + 1) * 8192)
                nc.sync.dma_start(b2r[:, sl], b2.ap()[g, q, :, :])

        # ---- load constants/weights into SBUF
        cw = {}
        for k in w:
            t = const.tile(list(w[k].shape), w[k].dtype, tag=f"cw_{k}")
            nc.sync.dma_start(t[:], w[k].ap())
            cw[k] = t
        for k, dte in (("cu_bias", cu_bias_d), ("lu_bias", lu_bias_d)):
            t = const.tile([DIM, 4], F32, tag=f"cw_{k}")
            nc.sync.dma_start(t[:], dte.ap().rearrange("g p -> p g"))
            cw[k] = t
        idt = const.tile([DIM, DIM], BF16, tag="idt")
        nc.sync.dma_start(idt[:], id128.ap())
        zbf = const.tile([DIM, 512], BF16, tag="zbf")
        nc.vector.memset(zbf[:], 0.0)
        lcbias = const.tile([DIM, CC], BF16, tag="lcbias")
        nc.sync.dma_start(lcbias[:], lcbias_d.ap())
        clbias = const.tile([DIM, LL], BF16, tag="clbias")
        nc.sync.dma_start(clbias[:], clbias_d.ap())

        # ---- persistent state (feature-major)
        lht = const.tile([DIM, LL], BF16, tag="lht")
        lct = const.tile([DIM, LL], BF16, tag="lct")
        cht = const.tile([DIM, CC], BF16, tag="cht")
        cct = const.tile([DIM, CC], BF16, tag="cct")
        nc.sync.dma_start(lht[:], lh0t.ap())
        nc.sync.dma_start(cht[:], ch0t.ap())
        nc.vector.memset(lct[:], 0.0)
        nc.vector.memset(cct[:], 0.0)

        def dma2(dst, src):
            """Split a [128, N] transfer across two DMA queues by partitions."""
            nc.sync.dma_start(dst[0:64, :], src[0:64, :])
            nc.sync.dma_start(dst[64:DIM, :], src[64:DIM, :])

        def mlp_chunk(x, pfx, sl, n, out_dt=BF16, tagsfx="", layers=(1, 2, 3)):
            """MLP layers on columns sl (chunks of <=512) of x [128, *]."""
            cur = x
            for li in layers:
                wt = cw[f"{pfx}_w{li}t"]
                bt = cw[f"{pfx}_b{li}"]
                m = wt.shape[1]
                o = sb.tile([m, n], out_dt if li == 3 else BF16, bufs=1,
                            tag=f"{pfx}_h{li}{tagsfx}", name=f"{pfx}_h{li}{tagsfx}")
                for rc in range(n // 512):
                    c0 = rc * 512
                    pt = ps.tile([m, 512], F32, tag="ps", name="mlp_ps")
                    src = cur[:, sl.start + c0:sl.start + c0 + 512] if li == layers[0] \
                        else cur[:, c0:c0 + 512]
                    nc.tensor.matmul(pt[:], wt[:], src, start=True, stop=True)
                    func = AF.Relu if li < 3 else AF.Identity
                    nc.scalar.activation(o[:, c0:c0 + 512], pt[:], func,
                                         bias=bt[:, 0:1])
                cur = o
            return cur

        def layer3_T(h2, pfx, n_tiles, dst, dst_off):
            """Transposed MLP layer 3: k-tile t of dst gets (h2_t)^T @ W3^T,
            i.e. L_pre^T/C_pre^T [128 rows, 128 feat] directly -- no PE
            transposes.  Layer-3 bias is folded into lcbias/clbias.  Psums
            are batched 4 k-tiles per [128,512] ring tile."""
            w3t = cw[f"{pfx}_w3t"]
            for t in range(n_tiles):
                pt = pstr.tile([DIM, DIM], F32, tag="pstr", name=f"{pfx}_l3t")
                nc.tensor.matmul(pt[:], h2[:, t * DIM:(t + 1) * DIM], w3t[:],
                                 start=True, stop=True)
                osl = slice((dst_off + t) * DIM, (dst_off + t + 1) * DIM)
                nc.vector.tensor_copy(dst[:, osl], pt[:])

        def lstm_elementwise(gps, bias, c_st, h_st, rc0, n):
            """gps: 4 psum tiles [128, n] (i,f,g,o); updates states [:, rc0:rc0+n]."""
            sl = slice(rc0, rc0 + n)
            sig_i = sb.tile([DIM, n], BF16, tag="lw_si", bufs=1, name="sig_i")
            sig_f = sb.tile([DIM, n], BF16, tag="lw_sf", bufs=1, name="sig_f")
            tng = sb.tile([DIM, n], BF16, tag="lw_tg", bufs=1, name="tng")
            sig_o = sb.tile([DIM, n], BF16, tag="lw_so", bufs=1, name="sig_o")
            nc.scalar.activation(sig_i[:], gps[0][:], AF.Sigmoid, bias=bias[:, 0:1])
            nc.scalar.activation(sig_f[:], gps[1][:], AF.Sigmoid, bias=bias[:, 1:2])
            nc.scalar.activation(tng[:], gps[2][:], AF.Tanh, bias=bias[:, 2:3])
            nc.scalar.activation(sig_o[:], gps[3][:], AF.Sigmoid, bias=bias[:, 3:4])
            t1 = sb.tile([DIM, n], BF16, tag="lw_t1", bufs=1, name="t1")
            nc.vector.tensor_mul(t1[:], sig_f[:], c_st[:, sl])
            t2 = sb.tile([DIM, n], BF16, tag="lw_t2", bufs=1, name="t2")
            nc.vector.tensor_mul(t2[:], sig_i[:], tng[:])
            nc.vector.tensor_add(c_st[:, sl], t1[:], t2[:])
            tnc = sb.tile([DIM, n], BF16, tag="lw_tc", bufs=1, name="tnc")
            nc.scalar.activation(tnc[:], c_st[:, sl], AF.Tanh)
            nc.vector.tensor_mul(h_st[:, sl], sig_o[:], tnc[:])

        def c_lstm(lct_ps):
            """C-LSTM over 4 clause chunks."""
            for rc in range(4):
                sl = slice(rc * 512, (rc + 1) * 512)
                lc_sb = sb.tile([DIM, 512], BF16, tag="lc_sb", bufs=2,
                                name=f"lc_sb{rc}")
                nc.vector.tensor_add(lc_sb[:], lct_ps[rc][:], lcbias[:, sl])
                gps = [ps.tile([DIM, 512], F32, tag="ps", name=f"cg{i}")
                       for i in range(4)]
                for g in range(4):
                    gsl = slice(g * DIM, (g + 1) * DIM)
                    nc.tensor.matmul(gps[g][:], cw["cu_wiht"][:, gsl],
                                     lc_sb[:], start=True, stop=False)
                    nc.tensor.matmul(gps[g][:], cw["cu_whht"][:, gsl],
                                     cht[:, sl], start=False, stop=True)
                lstm_elementwise(gps, cw["cu_bias"], cct, cht, rc * 512, 512)

        def c_mlp_half(ch, cpre_kt):
            """C_pre MLP (transposed layer 3) for clause half ch."""
            h2 = mlp_chunk(cht, "cmsg", slice(ch * 1024, (ch + 1) * 1024),
                           1024, layers=(1, 2))
            layer3_T(h2, "cmsg", 8, cpre_kt, ch * 8)

        N_B2_PRE = 3     # streamed-b2 window depth

        def b2_fetch(i):
            t = sb3.tile([DIM, 4 * 2048], FP8, tag="b2t", bufs=N_B2_PRE,
                         name=f"b2t{i}")
            nc.scalar.dma_start(t[:], b2.ap()[i // 4, i % 4, :, :])
            return t

        def gemm2_qrange(cpre_kt, g, cl_ps, b2s, qa, qb):
            """Accumulate GEMM2 group g over q-blocks [qa, qb)."""
            for q in range(qa, qb):
                blk = g * 4 + q
                if blk < N_B2_STREAM:
                    b2t = b2s[blk]
                    b2v = b2t[:].rearrange("p (t c) -> p t c", c=2048)
                else:
                    gsl = slice((blk - N_B2_STREAM) * 8192,
                                (blk - N_B2_STREAM + 1) * 8192)
                    b2v = b2r[:, gsl].rearrange("p (t c) -> p t c", c=2048)
                for kk in (0, 2):
                    k = 4 * q + kk
                    ck = cpre_kt[:, k * DIM:(k + 2) * DIM].rearrange(
                        "p (j d) -> p j d", j=2)
                    for i in range(4):
                        nc.tensor.matmul(
                            cl_ps[i][:], ck,
                            b2v[:, kk:kk + 2, i * 512:(i + 1) * 512],
                            start=(k == 0), stop=(k == KC - 2),
                            perf_mode=mybir.MatmulPerfMode.DoubleRow)
                if blk + N_B2_PRE < N_B2_STREAM:
                    b2s.append(b2_fetch(blk + N_B2_PRE))

        def gemm2_stage(g, cl_ps, rs_bufs):
            """Stage group g's 4 blocks with one contiguous DMA.  Groups 0/1
            copy on the (idle) vector engine so A2A_0 triggers sooner; groups
            2/3 stay on scalar to keep vector free for the A2A_0 reduce that
            runs concurrently."""
            h, b0 = (0, 0) if g == 0 else (0, 4) if g == 1 else \
                    (1, 0) if g == 2 else (1, 4)
            cs4 = sb.tile([DIM, 4 * 512], FP8, tag="cl_st", bufs=1,
                          name=f"cl_st{g}")
            for i in range(4):
                seg = cs4[:, i * 512:(i + 1) * 512]
                if g != 2:
                    # g0/g1: vector is idle, A2A_0 triggers sooner.  g3:
                    # vector frees its psums ~2us earlier, unblocking the
                    # L-gate warm-fill matmuls that reuse them (ring WAR).
                    nc.vector.tensor_copy(seg, cl_ps[i][:])
                else:
                    # g2 stays on scalar: its copies are closest to the
                    # A2A_0 reduce's vector window
                    nc.scalar.activation(seg, cl_ps[i][:], AF.Identity)
            dst = rs_bufs[h][b0 * DIM:(b0 + 4) * DIM, :].rearrange(
                "(b p) c -> p b c", p=DIM)
            src = cs4[:].rearrange("p (b c) -> p b c", c=512)
            nc.sync.dma_start(dst, src)

        def gemm2_psum(g):
            return [ps.tile([DIM, 512], F32, tag="ps", name=f"cl{g}_{i}")
                    for i in range(4)]

        def gemm2_group(cpre_kt, g, rs_bufs, r, b2s):
            """One GEMM2 group: 4 psum accums over KC k-tiles; stage to buf."""
            cl_ps = gemm2_psum(g)
            gemm2_qrange(cpre_kt, g, cl_ps, b2s, 0, 4)
            gemm2_stage(g, cl_ps, rs_bufs)

        def l_half(h, clt_h, lh_flip, r, n_warm, ag_in):
            """L-LSTM + L_pre MLP + transposes for local half h; returns ag_in."""
            sl = slice(h * 512, (h + 1) * 512)
            fsl = slice((1 - h) * 512, (2 - h) * 512)
            gps = [ps.tile([DIM, 512], F32, tag="ps", name=f"lg{h}_{i}")
                   for i in range(4)]
            # flip/hidden gate matmuls first: they only need lh_flip, so the
            # PE computes them while the A2A exchange is still in flight; the
            # clt matmuls (which wait on the reduce) come last.
            for g in range(4):
                gsl = slice(g * DIM, (g + 1) * DIM)
                for wi in range(n_warm):
                    nc.tensor.matmul(gps[g][:], idt[:], zbf[:],
                                     start=(wi == 0), stop=False)
                nc.tensor.matmul(gps[g][:], cw["lu_wiht_fl"][:, gsl],
                                 lh_flip[:, fsl], start=(n_warm == 0),
                                 stop=False)
                nc.tensor.matmul(gps[g][:], cw["lu_whht"][:, gsl],
                                 lh_flip[:, sl], start=False, stop=False)
            for g in range(4):
                gsl = slice(g * DIM, (g + 1) * DIM)
                for b in range(4):
                    nc.tensor.matmul(gps[g][:], cw["lu_wiht_cl"][:, gsl],
                                     clt_h[:, b * 512:(b + 1) * 512],
                                     start=False, stop=(b == 3))
            lstm_elementwise(gps, cw["lu_bias"], lct, lht, h * 512, 512)
            stage_lpre(h, ag_in)

        def stage_lpre(h, ag_in):
            """L_pre^T k-tiles for local half h -> ag_in rows [h*512:(h+1)*512]."""
            h2 = mlp_chunk(lht, "lmsg", slice(h * 512, (h + 1) * 512), 512,
                           tagsfx=f"_{h}", layers=(1, 2))
            lpt = sb.tile([DIM, 4 * DIM], FP8, tag=f"lpt_{h}", bufs=1,
                          name=f"lpt_{h}")
            layer3_T(h2, "lmsg", 4, lpt, 0)
            dst = ag_in[:].rearrange("(t p) d -> p t d", p=DIM)
            nc.sync.dma_start(dst, lpt[:].rearrange("p (t d) -> p t d", d=DIM))

        N_B1_PRE = 5     # b1 window depth (tile bufs / prologue prefetch)

        def b1_fetch(grp):
            t = sb3.tile([DIM, 4 * CC], FP8, tag="b1t", bufs=N_B1_PRE,
                         name=f"b1t{grp}")
            nc.scalar.dma_start(t[:], b1.ap()[grp, :, :])
            return t

        def gemm1_prologue():
            """Prefetch the first b1 groups; fires as the prior GEMM1 ends."""
            return [b1_fetch(j) for j in range(N_B1_PRE)]

        def gemm1(lpre_sb, n_warm, pre):
            """GEMM1: LC.T [128, 2048] psum accums over 64 packed k-tiles.

            Each group's refill DMA is issued right after the matmuls that
            free its buffer slot, so the scalar HWDGE ring never stalls."""
            tiles = list(pre)
            lct_ps = [ps.tile([DIM, 512], F32, tag="ps", name=f"g1_{i}")
                      for i in range(4)]
            for wi in range(n_warm):
                nc.tensor.matmul(lct_ps[wi % 4][:], idt[:], zbf[:],
                                 start=(wi < 4), stop=False)
            for grp in range(KL // 4):
                if grp == 8:
                    # bridge the AG half-1 wait without letting HAM cool
                    for wi in range(N_WARM_MID):
                        nc.tensor.matmul(lct_ps[wi % 4][:], idt[:], zbf[:],
                                         start=False, stop=False)
                b1t = tiles[grp]
                b1v = b1t[:].rearrange("p (t c) -> p t c", c=CC)
                for kk in (0, 2):
                    k = 4 * grp + kk
                    lf = lpre_sb[grp]
                    lk = lf[:, kk * DIM:(kk + 2) * DIM].rearrange(
                        "p (j d) -> p j d", j=2)
                    for c4 in range(4):
                        nc.tensor.matmul(
                            lct_ps[c4][:], lk,
                            b1v[:, kk:kk + 2, c4 * 512:(c4 + 1) * 512],
                            start=(k == 0 and n_warm == 0),
                            stop=(k == KL - 2),
                            perf_mode=mybir.MatmulPerfMode.DoubleRow)
                if grp + N_B1_PRE < KL // 4:
                    tiles.append(b1_fetch(grp + N_B1_PRE))
            return lct_ps

        def load_lpre(ag_outs):
            """Load AG halves as 16 per-group tiles of 4 k-tiles each.

            b1 groups are half-major [half, core, r]: group g = (half g//8,
            core g%8), so groups 0-7 depend only on AG half 0."""
            lpre_sb = []
            for g in range(16):
                h, c = g // 8, g % 8
                lt = sb.tile([DIM, 4 * DIM], FP8, tag="lpf", bufs=6,
                             name=f"lpf{g}")
                src = ag_outs[h][c * 512:(c + 1) * 512, :]
                s3 = src.rearrange("(t p) d -> p t d", p=DIM)
                d3 = lt[:].rearrange("p (t d) -> p t d", d=DIM)
                nc.sync.dma_start(d3, s3)
                lpre_sb.append(lt)
            return lpre_sb

        rg = [list(range(N_CORES))]

        def collective(kind, op, cin, cout):
            nc.gpsimd.collective_compute(kind, op, replica_groups=rg,
                                         ins=[cin.opt()], outs=[cout.opt()])

        # ====== round 0 head: L_pre^T from Lh0 -> ag_in halves ======
        ag_ins = []
        for h in range(2):
            ag_in = dram.tile([512, DIM], FP8, tag=f"ag_in{h}",
                              name=f"ag_in{h}_init")
            stage_lpre(h, ag_in)
            ag_ins.append(ag_in)

        def a2a_load(h, ro, r):
            """Load A2A output (8 partial blocks) into SBUF.

            Issued for BOTH halves before l_half(0), so half 1's load fires
            the moment A2A_1 completes instead of queuing on the sync ring
            behind lhalf0's AG staging DMA.  Two half-loads per buffer let
            the first pair-sums start while blocks 4-7 are in flight."""
            a2a_sb = sb.tile([DIM, 8 * 512], FP8, tag="a2a_sb", bufs=1,
                             name=f"a2a_sb{h}_{r}")
            src3 = ro[:].rearrange("(b p) c -> p b c", p=DIM)
            dst3 = a2a_sb[:].rearrange("p (b c) -> p b c", c=512)
            nc.sync.dma_start(dst3[:, 0:4], src3[:, 0:4])
            nc.sync.dma_start(dst3[:, 4:8], src3[:, 4:8])
            return a2a_sb

        def a2a_sum(h, a2a_sb, r):
            """Pair-sum the 8 partials -> 4 bf16 partials (exact in bf16).

            The remaining two reduce levels happen for free inside the
            L-LSTM input matmuls (psum-accumulated), on the idle PE."""
            blk = lambda b: a2a_sb[:, b * 512:(b + 1) * 512]
            s1 = sb.tile([DIM, 4 * 512], BF16, tag="a2a_s1", bufs=1,
                         name=f"s1_{h}_{r}")
            for b in range(4):
                nc.vector.tensor_add(s1[:, b * 512:(b + 1) * 512],
                                     blk(2 * b), blk(2 * b + 1))
            # fold deg(lit)*Cmsg_b3 into branch 0 (off the reduce critical path)
            nc.vector.tensor_add(s1[:, 0:512], s1[:, 0:512],
                                 clbias[:, h * 512:(h + 1) * 512])
            return s1

        b1pre = gemm1_prologue()
        for r in range(N_ROUNDS):
            ag_outs = []
            for h in range(2):
                ag_out = dram.tile([4096, DIM], FP8, tag=f"ag_out{h}",
                                   addr_space="Shared", name=f"ag_out{h}_{r}")
                collective("AllGather", mybir.AluOpType.bypass,
                           ag_ins[h], ag_out)
                ag_outs.append(ag_out)
            lpre_sb = load_lpre(ag_outs)
            lct_ps = gemm1(lpre_sb, N_WARM_G1, b1pre)
            if r + 1 < N_ROUNDS:
                b1pre = gemm1_prologue()
            b2s = [b2_fetch(i) for i in range(min(N_B2_STREAM, N_B2_PRE))]

            c_lstm(lct_ps)
            cpre_kt = sb.tile([DIM, KC * DIM], FP8, tag="cpre_kt", bufs=1,
                              name="cpre_kt")
            c_mlp_half(0, cpre_kt)
            c_mlp_half(1, cpre_kt)

            lh_flip = sb.tile([DIM, LL], BF16, tag="lh_flip", bufs=1,
                              name="lh_flip")
            nc.scalar.activation(lh_flip[:], lht[:], AF.Identity)

            rs_bufs = [dram.tile([N_CORES * DIM, 512], FP8, tag=f"rs_in{h}",
                                 name=f"rs_in{h}_{r}") for h in range(2)]
            gemm2_group(cpre_kt, 0, rs_bufs, r, b2s)
            gemm2_group(cpre_kt, 1, rs_bufs, r, b2s)
            ro0 = dram.tile([N_CORES * DIM, 512], FP8, tag="rs_out0",
                            name=f"rs_out0_{r}")
            collective("AllToAll", mybir.AluOpType.bypass, rs_bufs[0], ro0)
            gemm2_group(cpre_kt, 2, rs_bufs, r, b2s)
            gemm2_group(cpre_kt, 3, rs_bufs, r, b2s)

            ro1 = dram.tile([N_CORES * DIM, 512], FP8, tag="rs_out1",
                            name=f"rs_out1_{r}")
            collective("AllToAll", mybir.AluOpType.bypass, rs_bufs[1], ro1)

            ag_ins = [dram.tile([512, DIM], FP8, tag=f"ag_in{h}",
                                name=f"ag_in{h}_{r}") for h in range(2)]
            sb0 = a2a_load(0, ro0, r)
            sb1 = a2a_load(1, ro1, r)
            clt0 = a2a_sum(0, sb0, r)
            l_half(0, clt0, lh_flip, r, N_WARM1, ag_ins[0])

            clt1 = a2a_sum(1, sb1, r)
            l_half(1, clt1, lh_flip, r, N_WARM2, ag_ins[1])

        # ---- vote MLP on final Lh -> [1, 1024] f32
        vt0 = mlp_chunk(lht, "lvote", slice(0, 512), 512, out_dt=F32,
                        tagsfx="_0")
        vt1 = mlp_chunk(lht, "lvote", slice(512, 1024), 512, out_dt=F32,
                        tagsfx="_1")
        nc.sync.dma_start(vote_out.ap()[:, 0:512], vt0[:])
        nc.sync.dma_start(vote_out.ap()[:, 512:1024], vt1[:])

    nc.compile()
    _CACHE["nc"] = nc
    return nc


def _perm_rows(lits):
    """Map global lit index -> permuted row (core-major, 1024 rows/core)."""
    lits = np.asarray(lits)
    neg = lits >= N_VARS
    v = np.where(neg, lits - N_VARS, lits)
    core = v // VPC
    r = v % VPC
    return core * LL + np.where(neg, VPAD + r, r)


def _b1_row_order():
    """B1 rows: [half h, core c, r] -> permuted row c*1024 + h*512 + r."""
    order = np.empty(LPAD, np.int64)
    n = 0
    for h in range(2):
        for c in range(N_CORES):
            order[n:n + 512] = c * LL + h * 512 + np.arange(512)
            n += 512
    return order


def host_prep(inp):
    f32 = np.float32
    idx = inp["L_unpack_indices"].astype(np.int64)
    rows = _perm_rows(idx[:, 0])
    M = np.zeros((LPAD, CPAD), np.float32)
    np.add.at(M, (rows, idx[:, 1]), 1.0)

    # degree vectors for the folded layer-3 biases
    deg_c = M.sum(axis=0)                        # [CPAD] clause degrees
    deg_l = M.sum(axis=1)                        # [LPAD] permuted lit degrees
    lmsg_b3 = inp["Lmsg_b3"].astype(f32)
    cmsg_b3 = inp["Cmsg_b3"].astype(f32)

    row_order = _b1_row_order()
    b1s, b2s, lcbs, clbs = [], [], [], []
    for i in range(N_CORES):
        blk = M[:, i * CC:(i + 1) * CC]          # [8192, 2048] permuted rows
        b1o = blk[row_order]                      # AG-concat row order
        # pack 4 k-tiles per DMA group: [16, 128, 4*2048]
        b1p = b1o.reshape(16, 4, DIM, CC).transpose(0, 2, 1, 3) \
                 .reshape(16, DIM, 4 * CC)
        b1s.append(np.ascontiguousarray(b1p).astype(nf8))
        bT = blk.T                                # [2048 clauses, 8192 lits]
        grp = []
        for g in range(4):
            cols = np.concatenate([np.arange(j * 512, (j + 1) * 512)
                                   for j in J_SETS[g]])
            gb = bT[:, cols]                      # [2048, 2048]
            gp = gb.reshape(4, 4, DIM, 2048).transpose(0, 2, 1, 3) \
                   .reshape(4, DIM, 4 * 2048)
            grp.append(gp)
        b2s.append(np.ascontiguousarray(np.stack(grp)).astype(nf8))
        lcbs.append(np.outer(lmsg_b3, deg_c[i * CC:(i + 1) * CC]))
        clbs.append(np.outer(cmsg_b3, deg_l[i * LL:(i + 1) * LL]))

    def bf(x):
        return np.ascontiguousarray(x).astype(nbf)

    l0 = (inp["L_init_w"][:, 0] + inp["L_init_b"]).astype(f32)
    c0 = (inp["C_init_w"][:, 0] + inp["C_init_b"]).astype(f32)
    common = {
        "lh0t": bf(np.repeat(l0[:, None], LL, axis=1)),
        "ch0t": bf(np.repeat(c0[:, None], CC, axis=1)),
        "id128": bf(np.eye(DIM, dtype=f32)),
        "cu_wiht": bf(inp["Cu_wih"].T), "cu_whht": bf(inp["Cu_whh"].T),
        "lu_wiht_cl": bf(inp["Lu_wih"].T[:DIM]),
        "lu_wiht_fl": bf(inp["Lu_wih"].T[DIM:]),
        "lu_whht": bf(inp["Lu_whh"].T),
        "cu_bias": (inp["Cu_bih"] + inp["Cu_bhh"]).astype(f32).reshape(4, DIM),
        "lu_bias": (inp["Lu_bih"] + inp["Lu_bhh"]).astype(f32).reshape(4, DIM),
    }
    for p, P in (("lmsg", "Lmsg"), ("cmsg", "Cmsg"), ("lvote", "Lvote")):
        for i in (1, 2, 3):
            common[f"{p}_w{i}t"] = bf(inp[f"{P}_w{i}"].T)
            bshape = (1, 1) if (p == "lvote" and i == 3) else (DIM, 1)
            common[f"{p}_b{i}"] = inp[f"{P}_b{i}"].astype(f32).reshape(bshape)
    return [dict(common, b1=b1s[i], b2=b2s[i], lcbias=bf(lcbs[i]),
                 clbias=bf(clbs[i])) for i in range(N_CORES)]


def kernel(**inputs):
    inp = {k: np.asarray(v) for k, v in inputs.items()}
    in_maps = host_prep(inp)
    nc = _build()
    res = bass_utils.run_bass_kernel_spmd(nc, in_maps,
                                          core_ids=list(range(N_CORES)))
    probs = np.zeros(N_CORES, np.float32)
    for i in range(N_CORES):
        v = res.results[i]["vote"][0]            # [1024]
        s = v[:VPC].astype(np.float64).sum() + \
            v[VPAD:VPAD + VPC].astype(np.float64).sum()
        probs[i] = np.float32(s / (2 * VPC))
    return probs



# revision 133
# speedup vs baseline: 1.0522x; 1.0117x over previous
"""NeuroSAT message-passing GNN on 8 TRN2 NeuronCores (Bass/Tile).

Sharding: clause dim sharded 8-way (2048 padded clauses/core); literal dim
permuted so core i owns problem i's 500 vars (+12 pads) as 1024 lit rows
(512 pos + 512 neg).  Per round (pipelined):
  GEMM2 groups 0,1 -> AllToAll half0 ; groups 2,3 -> AllToAll half1 (fp8)
  partials summed locally on DVE (f32), L-LSTM + L_pre MLP per half,
  AllGather halves of L_pre (fp8, Shared-output Mesh)
  GEMM1 LC.T = L_pre.T @ B1 ; C-LSTM ; C_pre MLP ; repeat
M (counts) is exact in fp8e4m3; fp8 M blocks stream from HBM as the moving
operand against fp8 stationary activations (DoubleRow).  AllToAll is used
instead of ReduceScatter because it always runs the O(1)-hop Mesh algorithm
(RS picks RDH at this size: ~2x slower); the 8 partial blocks are reduced
on the vector engine.  Zero-contribution dummy matmuls keep the PE HAM-warm
(K=8/8 clock) across the residual collective waits.
"""

import numpy as np
import ml_dtypes

import concourse.bass as bass
import concourse.bacc as bacc
import concourse.mybir as mybir
import concourse.tile as tile
from concourse import bass_utils

F32 = mybir.dt.float32
BF16 = mybir.dt.bfloat16
FP8 = mybir.dt.float8e4
AF = mybir.ActivationFunctionType

N_CORES = 8
DIM = 128
N_ROUNDS = 16
N_VARS = 4000
VPC = 500            # real vars per core (= vars per problem)
VPAD = 512           # padded vars per core
LL = 2 * VPAD        # 1024 lit rows per core
LPAD = N_CORES * LL  # 8192
CC = 2048            # padded clauses per core
CPAD = N_CORES * CC  # 16384
KL = LPAD // 128     # 64 k-tiles over lits
KC = CC // 128       # 16 k-tiles over clauses

# GEMM2 groups: group g computes 512-lit chunks J_SETS[g]; chunk j covers
# local lit rows [512*(j%2)...) of destination core j//2.  Groups 0,1 cover
# all even j (RS half 0 = every core's rows 0:512); groups 2,3 odd j.
J_SETS = [[0, 2, 4, 6], [8, 10, 12, 14], [1, 3, 5, 7], [9, 11, 13, 15]]

N_WARM1 = 8          # dummy MMs per gate group, L half 0 (A2A_0 wait)
N_WARM_G1 = 12       # dummy MM prefix on GEMM1 (AG + load window)
N_WARM_MID = 0       # dummy MMs between GEMM1 halves (AG_1 tail)
N_WARM2 = 6          # dummy MMs per gate group, L half 1 (A2A_1 wait)
N_B2_STREAM = 7      # streamed b2 blocks 0..6 (g0/g1); 7..15 stay resident.
                     # The streamed blocks belong to GEMM2's FIRST groups so
                     # all HBM streaming finishes before A2A_0 fires; g2/g3
                     # (which run inside the A2A windows) touch only SBUF,
                     # leaving HBM quiet for the collectives.

nbf = ml_dtypes.bfloat16
nf8 = ml_dtypes.float8_e4m3

_CACHE = {}


def _build():
    """Build + compile the SPMD program once (shape-only, no input values)."""
    if "nc" in _CACHE:
        return _CACHE["nc"]

    nc = bacc.Bacc("TRN2", target_bir_lowering=False, debug=False,
                   num_devices=N_CORES)

    def din(name, shape, dt):
        return nc.dram_tensor(name, shape, dt, kind="ExternalInput")

    # b1: 16 packed groups of 4 k-tiles; rows ordered [half h, core c, r<512]
    b1 = din("b1", [KL // 4, DIM, 4 * CC], FP8)
    # b2[g]: group g's 16 k-tiles packed 4-per-DMA: [4 groups, 4 qgrp, 128, 4*2048]
    b2 = din("b2", [4, 4, DIM, 4 * 2048], FP8)
    lh0t = din("lh0t", [DIM, LL], BF16)
    ch0t = din("ch0t", [DIM, CC], BF16)
    id128 = din("id128", [DIM, DIM], BF16)
    # folded layer-3 biases: LC += deg(clause)*Lmsg_b3, CL += deg(lit)*Cmsg_b3
    lcbias_d = din("lcbias", [DIM, CC], BF16)
    clbias_d = din("clbias", [DIM, LL], BF16)

    w = {}
    for p in ("lmsg", "cmsg", "lvote"):
        for i in (1, 2, 3):
            shp = [DIM, 1] if (p == "lvote" and i == 3) else [DIM, DIM]
            w[f"{p}_w{i}t"] = din(f"{p}_w{i}t", shp, BF16)
            bshp = [1, 1] if (p == "lvote" and i == 3) else [DIM, 1]
            w[f"{p}_b{i}"] = din(f"{p}_b{i}", bshp, F32)
    w["cu_wiht"] = din("cu_wiht", [DIM, 4 * DIM], BF16)
    w["cu_whht"] = din("cu_whht", [DIM, 4 * DIM], BF16)
    w["lu_wiht_cl"] = din("lu_wiht_cl", [DIM, 4 * DIM], BF16)
    w["lu_wiht_fl"] = din("lu_wiht_fl", [DIM, 4 * DIM], BF16)
    w["lu_whht"] = din("lu_whht", [DIM, 4 * DIM], BF16)
    cu_bias_d = din("cu_bias", [4, DIM], F32)
    lu_bias_d = din("lu_bias", [4, DIM], F32)

    vote_out = nc.dram_tensor("vote", [1, LL], F32, kind="ExternalOutput")

    with tile.TileContext(nc) as tc, \
         tc.tile_pool(name="const", bufs=1) as const, \
         tc.tile_pool(name="sb", bufs=2) as sb, \
         tc.tile_pool(name="sb3", bufs=2) as sb3, \
         tc.tile_pool(name="ps", bufs=6, space="PSUM") as ps, \
         tc.tile_pool(name="pstr", bufs=2, space="PSUM") as pstr, \
         tc.tile_pool(name="dram", bufs=2, space="DRAM") as dram:

        # ---- resident b2: blocks N_B2_STREAM..15 of [128, 8192] fp8.
        # Loads issued later (scalar ring, behind the b1 prologue) so round
        # 0's weight loads on the sync ring aren't queued behind 9.4MB.
        b2r = const.tile([DIM, (16 - N_B2_STREAM) * 4 * 2048], FP8, tag="b2r")

        def b2r_init():
            for g in range(4):
                for q in range(4):
                    blk = g * 4 + q
                    if blk < N_B2_STREAM:
                        continue
                    sl = slice((blk - N_B2_STREAM) * 8192,
                               (blk - N_B2_STREAM + 1) * 8192)
                    nc.scalar.dma_start(b2r[:, sl], b2.ap()[g, q, :, :])

        # ---- load constants/weights into SBUF
        cw = {}
        for k in w:
            t = const.tile(list(w[k].shape), w[k].dtype, tag=f"cw_{k}")
            nc.sync.dma_start(t[:], w[k].ap())
            cw[k] = t
        for k, dte in (("cu_bias", cu_bias_d), ("lu_bias", lu_bias_d)):
            t = const.tile([DIM, 4], F32, tag=f"cw_{k}")
            nc.sync.dma_start(t[:], dte.ap().rearrange("g p -> p g"))
            cw[k] = t
        idt = const.tile([DIM, DIM], BF16, tag="idt")
        nc.sync.dma_start(idt[:], id128.ap())
        zbf = const.tile([DIM, 512], BF16, tag="zbf")
        nc.vector.memset(zbf[:], 0.0)
        lcbias = const.tile([DIM, CC], BF16, tag="lcbias")
        nc.sync.dma_start(lcbias[:], lcbias_d.ap())
        clbias = const.tile([DIM, LL], BF16, tag="clbias")
        nc.sync.dma_start(clbias[:], clbias_d.ap())

        # ---- persistent state (feature-major)
        lht = const.tile([DIM, LL], BF16, tag="lht")
        lct = const.tile([DIM, LL], BF16, tag="lct")
        cht = const.tile([DIM, CC], BF16, tag="cht")
        cct = const.tile([DIM, CC], BF16, tag="cct")
        nc.sync.dma_start(lht[:], lh0t.ap())
        nc.sync.dma_start(cht[:], ch0t.ap())
        nc.vector.memset(lct[:], 0.0)
        nc.vector.memset(cct[:], 0.0)

        def dma2(dst, src):
            """Split a [128, N] transfer across two DMA queues by partitions."""
            nc.sync.dma_start(dst[0:64, :], src[0:64, :])
            nc.sync.dma_start(dst[64:DIM, :], src[64:DIM, :])

        def mlp_chunk(x, pfx, sl, n, out_dt=BF16, tagsfx="", layers=(1, 2, 3)):
            """MLP layers on columns sl (chunks of <=512) of x [128, *]."""
            cur = x
            for li in layers:
                wt = cw[f"{pfx}_w{li}t"]
                bt = cw[f"{pfx}_b{li}"]
                m = wt.shape[1]
                o = sb.tile([m, n], out_dt if li == 3 else BF16, bufs=1,
                            tag=f"{pfx}_h{li}{tagsfx}", name=f"{pfx}_h{li}{tagsfx}")
                for rc in range(n // 512):
                    c0 = rc * 512
                    pt = ps.tile([m, 512], F32, tag="ps", name="mlp_ps")
                    src = cur[:, sl.start + c0:sl.start + c0 + 512] if li == layers[0] \
                        else cur[:, c0:c0 + 512]
                    nc.tensor.matmul(pt[:], wt[:], src, start=True, stop=True)
                    func = AF.Relu if li < 3 else AF.Identity
                    nc.scalar.activation(o[:, c0:c0 + 512], pt[:], func,
                                         bias=bt[:, 0:1])
                cur = o
            return cur

        def layer3_T(h2, pfx, n_tiles, dst, dst_off):
            """Transposed MLP layer 3: k-tile t of dst gets (h2_t)^T @ W3^T,
            i.e. L_pre^T/C_pre^T [128 rows, 128 feat] directly -- no PE
            transposes.  Layer-3 bias is folded into lcbias/clbias.  Psums
            are batched 4 k-tiles per [128,512] ring tile."""
            w3t = cw[f"{pfx}_w3t"]
            for t in range(n_tiles):
                pt = pstr.tile([DIM, DIM], F32, tag="pstr", name=f"{pfx}_l3t")
                nc.tensor.matmul(pt[:], h2[:, t * DIM:(t + 1) * DIM], w3t[:],
                                 start=True, stop=True)
                osl = slice((dst_off + t) * DIM, (dst_off + t + 1) * DIM)
                nc.vector.tensor_copy(dst[:, osl], pt[:])

        def lstm_elementwise(gps, bias, c_st, h_st, rc0, n):
            """gps: 4 psum tiles [128, n] (i,f,g,o); updates states [:, rc0:rc0+n]."""
            sl = slice(rc0, rc0 + n)
            sig_i = sb.tile([DIM, n], BF16, tag="lw_si", bufs=1, name="sig_i")
            sig_f = sb.tile([DIM, n], BF16, tag="lw_sf", bufs=1, name="sig_f")
            tng = sb.tile([DIM, n], BF16, tag="lw_tg", bufs=1, name="tng")
            sig_o = sb.tile([DIM, n], BF16, tag="lw_so", bufs=1, name="sig_o")
            nc.scalar.activation(sig_i[:], gps[0][:], AF.Sigmoid, bias=bias[:, 0:1])
            nc.scalar.activation(sig_f[:], gps[1][:], AF.Sigmoid, bias=bias[:, 1:2])
            nc.scalar.activation(tng[:], gps[2][:], AF.Tanh, bias=bias[:, 2:3])
            nc.scalar.activation(sig_o[:], gps[3][:], AF.Sigmoid, bias=bias[:, 3:4])
            t1 = sb.tile([DIM, n], BF16, tag="lw_t1", bufs=1, name="t1")
            nc.vector.tensor_mul(t1[:], sig_f[:], c_st[:, sl])
            t2 = sb.tile([DIM, n], BF16, tag="lw_t2", bufs=1, name="t2")
            nc.vector.tensor_mul(t2[:], sig_i[:], tng[:])
            nc.vector.tensor_add(c_st[:, sl], t1[:], t2[:])
            tnc = sb.tile([DIM, n], BF16, tag="lw_tc", bufs=1, name="tnc")
            nc.scalar.activation(tnc[:], c_st[:, sl], AF.Tanh)
            nc.vector.tensor_mul(h_st[:, sl], sig_o[:], tnc[:])

        def c_lstm(lct_ps):
            """C-LSTM over 4 clause chunks."""
            for rc in range(4):
                sl = slice(rc * 512, (rc + 1) * 512)
                lc_sb = sb.tile([DIM, 512], BF16, tag="lc_sb", bufs=2,
                                name=f"lc_sb{rc}")
                nc.vector.tensor_add(lc_sb[:], lct_ps[rc][:], lcbias[:, sl])
                gps = [ps.tile([DIM, 512], F32, tag="ps", name=f"cg{i}")
                       for i in range(4)]
                for g in range(4):
                    gsl = slice(g * DIM, (g + 1) * DIM)
                    nc.tensor.matmul(gps[g][:], cw["cu_wiht"][:, gsl],
                                     lc_sb[:], start=True, stop=False)
                    nc.tensor.matmul(gps[g][:], cw["cu_whht"][:, gsl],
                                     cht[:, sl], start=False, stop=True)
                lstm_elementwise(gps, cw["cu_bias"], cct, cht, rc * 512, 512)

        def c_mlp_half(ch, cpre_kt):
            """C_pre MLP (transposed layer 3) for clause half ch."""
            h2 = mlp_chunk(cht, "cmsg", slice(ch * 1024, (ch + 1) * 1024),
                           1024, layers=(1, 2))
            layer3_T(h2, "cmsg", 8, cpre_kt, ch * 8)

        N_B2_PRE = 3     # streamed-b2 window depth

        def b2_fetch(i):
            t = sb3.tile([DIM, 4 * 2048], FP8, tag="b2t", bufs=N_B2_PRE,
                         name=f"b2t{i}")
            nc.scalar.dma_start(t[:], b2.ap()[i // 4, i % 4, :, :])
            return t

        def gemm2_qrange(cpre_kt, g, cl_ps, b2s, qa, qb):
            """Accumulate GEMM2 group g over q-blocks [qa, qb)."""
            for q in range(qa, qb):
                blk = g * 4 + q
                if blk < N_B2_STREAM:
                    b2t = b2s[blk]
                    b2v = b2t[:].rearrange("p (t c) -> p t c", c=2048)
                else:
                    gsl = slice((blk - N_B2_STREAM) * 8192,
                                (blk - N_B2_STREAM + 1) * 8192)
                    b2v = b2r[:, gsl].rearrange("p (t c) -> p t c", c=2048)
                for kk in (0, 2):
                    k = 4 * q + kk
                    ck = cpre_kt[:, k * DIM:(k + 2) * DIM].rearrange(
                        "p (j d) -> p j d", j=2)
                    for i in range(4):
                        nc.tensor.matmul(
                            cl_ps[i][:], ck,
                            b2v[:, kk:kk + 2, i * 512:(i + 1) * 512],
                            start=(k == 0), stop=(k == KC - 2),
                            perf_mode=mybir.MatmulPerfMode.DoubleRow)
                if blk + N_B2_PRE < N_B2_STREAM:
                    b2s.append(b2_fetch(blk + N_B2_PRE))

        def gemm2_stage(g, cl_ps, rs_bufs):
            """Stage group g's 4 blocks with one contiguous DMA.  Groups 0/1
            copy on the (idle) vector engine so A2A_0 triggers sooner; groups
            2/3 stay on scalar to keep vector free for the A2A_0 reduce that
            runs concurrently."""
            h, b0 = (0, 0) if g == 0 else (0, 4) if g == 1 else \
                    (1, 0) if g == 2 else (1, 4)
            cs4 = sb.tile([DIM, 4 * 512], FP8, tag="cl_st", bufs=1,
                          name=f"cl_st{g}")
            for i in range(4):
                seg = cs4[:, i * 512:(i + 1) * 512]
                if g != 2:
                    # g0/g1: vector is idle, A2A_0 triggers sooner.  g3:
                    # vector frees its psums ~2us earlier, unblocking the
                    # L-gate warm-fill matmuls that reuse them (ring WAR).
                    nc.vector.tensor_copy(seg, cl_ps[i][:])
                else:
                    # g2 stays on scalar: its copies are closest to the
                    # A2A_0 reduce's vector window
                    nc.scalar.activation(seg, cl_ps[i][:], AF.Identity)
            dst = rs_bufs[h][b0 * DIM:(b0 + 4) * DIM, :].rearrange(
                "(b p) c -> p b c", p=DIM)
            src = cs4[:].rearrange("p (b c) -> p b c", c=512)
            nc.sync.dma_start(dst, src)

        def gemm2_psum(g):
            return [ps.tile([DIM, 512], F32, tag="ps", name=f"cl{g}_{i}")
                    for i in range(4)]

        def gemm2_group(cpre_kt, g, rs_bufs, r, b2s):
            """One GEMM2 group: 4 psum accums over KC k-tiles; stage to buf."""
            cl_ps = gemm2_psum(g)
            gemm2_qrange(cpre_kt, g, cl_ps, b2s, 0, 4)
            gemm2_stage(g, cl_ps, rs_bufs)

        def l_half(h, clt_h, lh_flip, r, n_warm, ag_in):
            """L-LSTM + L_pre MLP + transposes for local half h; returns ag_in."""
            sl = slice(h * 512, (h + 1) * 512)
            fsl = slice((1 - h) * 512, (2 - h) * 512)
            gps = [ps.tile([DIM, 512], F32, tag="ps", name=f"lg{h}_{i}")
                   for i in range(4)]
            # flip/hidden gate matmuls first: they only need lh_flip, so the
            # PE computes them while the A2A exchange is still in flight; the
            # clt matmuls (which wait on the reduce) come last.
            for g in range(4):
                gsl = slice(g * DIM, (g + 1) * DIM)
                for wi in range(n_warm):
                    nc.tensor.matmul(gps[g][:], idt[:], zbf[:],
                                     start=(wi == 0), stop=False)
                nc.tensor.matmul(gps[g][:], cw["lu_wiht_fl"][:, gsl],
                                 lh_flip[:, fsl], start=(n_warm == 0),
                                 stop=False)
                nc.tensor.matmul(gps[g][:], cw["lu_whht"][:, gsl],
                                 lh_flip[:, sl], start=False, stop=False)
            for g in range(4):
                gsl = slice(g * DIM, (g + 1) * DIM)
                for b in range(4):
                    nc.tensor.matmul(gps[g][:], cw["lu_wiht_cl"][:, gsl],
                                     clt_h[:, b * 512:(b + 1) * 512],
                                     start=False, stop=(b == 3))
            lstm_elementwise(gps, cw["lu_bias"], lct, lht, h * 512, 512)
            if ag_in is not None:
                stage_lpre(h, ag_in)

        def stage_lpre(h, ag_in):
            """L_pre^T k-tiles for local half h -> ag_in rows [h*512:(h+1)*512]."""
            h2 = mlp_chunk(lht, "lmsg", slice(h * 512, (h + 1) * 512), 512,
                           tagsfx=f"_{h}", layers=(1, 2))
            lpt = sb.tile([DIM, 4 * DIM], FP8, tag=f"lpt_{h}", bufs=1,
                          name=f"lpt_{h}")
            layer3_T(h2, "lmsg", 4, lpt, 0)
            dst = ag_in[:].rearrange("(t p) d -> p t d", p=DIM)
            nc.sync.dma_start(dst, lpt[:].rearrange("p (t d) -> p t d", d=DIM))

        N_B1_PRE = 5     # b1 window depth (tile bufs / prologue prefetch)

        def b1_fetch(grp):
            t = sb3.tile([DIM, 4 * CC], FP8, tag="b1t", bufs=N_B1_PRE,
                         name=f"b1t{grp}")
            nc.scalar.dma_start(t[:], b1.ap()[grp, :, :])
            return t

        def gemm1_prologue():
            """Prefetch the first b1 groups; fires as the prior GEMM1 ends."""
            return [b1_fetch(j) for j in range(N_B1_PRE)]

        def gemm1(lpre_sb, n_warm, pre):
            """GEMM1: LC.T [128, 2048] psum accums over 64 packed k-tiles.

            Each group's refill DMA is issued right after the matmuls that
            free its buffer slot, so the scalar HWDGE ring never stalls."""
            tiles = list(pre)
            lct_ps = [ps.tile([DIM, 512], F32, tag="ps", name=f"g1_{i}")
                      for i in range(4)]
            for wi in range(n_warm):
                nc.tensor.matmul(lct_ps[wi % 4][:], idt[:], zbf[:],
                                 start=(wi < 4), stop=False)
            for grp in range(KL // 4):
                if grp == 8:
                    # bridge the AG half-1 wait without letting HAM cool
                    for wi in range(N_WARM_MID):
                        nc.tensor.matmul(lct_ps[wi % 4][:], idt[:], zbf[:],
                                         start=False, stop=False)
                b1t = tiles[grp]
                b1v = b1t[:].rearrange("p (t c) -> p t c", c=CC)
                for kk in (0, 2):
                    k = 4 * grp + kk
                    lf = lpre_sb[grp]
                    lk = lf[:, kk * DIM:(kk + 2) * DIM].rearrange(
                        "p (j d) -> p j d", j=2)
                    for c4 in range(4):
                        nc.tensor.matmul(
                            lct_ps[c4][:], lk,
                            b1v[:, kk:kk + 2, c4 * 512:(c4 + 1) * 512],
                            start=(k == 0 and n_warm == 0),
                            stop=(k == KL - 2),
                            perf_mode=mybir.MatmulPerfMode.DoubleRow)
                if grp + N_B1_PRE < KL // 4:
                    tiles.append(b1_fetch(grp + N_B1_PRE))
            return lct_ps

        def load_lpre(ag_outs):
            """Load AG halves as 16 per-group tiles of 4 k-tiles each.

            b1 groups are half-major [half, core, r]: group g = (half g//8,
            core g%8), so groups 0-7 depend only on AG half 0."""
            lpre_sb = []
            for g in range(16):
                h, c = g // 8, g % 8
                lt = sb.tile([DIM, 4 * DIM], FP8, tag="lpf", bufs=6,
                             name=f"lpf{g}")
                src = ag_outs[h][c * 512:(c + 1) * 512, :]
                s3 = src.rearrange("(t p) d -> p t d", p=DIM)
                d3 = lt[:].rearrange("p (t d) -> p t d", d=DIM)
                nc.sync.dma_start(d3, s3)
                lpre_sb.append(lt)
            return lpre_sb

        rg = [list(range(N_CORES))]

        def collective(kind, op, cin, cout):
            nc.gpsimd.collective_compute(kind, op, replica_groups=rg,
                                         ins=[cin.opt()], outs=[cout.opt()])

        # ====== round 0 head: L_pre^T from Lh0 -> ag_in halves ======
        ag_ins = []
        for h in range(2):
            ag_in = dram.tile([512, DIM], FP8, tag=f"ag_in{h}",
                              name=f"ag_in{h}_init")
            stage_lpre(h, ag_in)
            ag_ins.append(ag_in)

        def a2a_load(h, ro, r):
            """Load A2A output (8 partial blocks) into SBUF.

            Issued for BOTH halves before l_half(0), so half 1's load fires
            the moment A2A_1 completes instead of queuing on the sync ring
            behind lhalf0's AG staging DMA.  Two half-loads per buffer let
            the first pair-sums start while blocks 4-7 are in flight."""
            a2a_sb = sb.tile([DIM, 8 * 512], FP8, tag="a2a_sb", bufs=1,
                             name=f"a2a_sb{h}_{r}")
            src3 = ro[:].rearrange("(b p) c -> p b c", p=DIM)
            dst3 = a2a_sb[:].rearrange("p (b c) -> p b c", c=512)
            nc.sync.dma_start(dst3[:, 0:4], src3[:, 0:4])
            nc.sync.dma_start(dst3[:, 4:8], src3[:, 4:8])
            return a2a_sb

        def a2a_sum(h, a2a_sb, r):
            """Pair-sum the 8 partials -> 4 bf16 partials (exact in bf16).

            The remaining two reduce levels happen for free inside the
            L-LSTM input matmuls (psum-accumulated), on the idle PE."""
            blk = lambda b: a2a_sb[:, b * 512:(b + 1) * 512]
            s1 = sb.tile([DIM, 4 * 512], BF16, tag="a2a_s1", bufs=1,
                         name=f"s1_{h}_{r}")
            for b in range(4):
                nc.vector.tensor_add(s1[:, b * 512:(b + 1) * 512],
                                     blk(2 * b), blk(2 * b + 1))
            # fold deg(lit)*Cmsg_b3 into branch 0 (off the reduce critical path)
            nc.vector.tensor_add(s1[:, 0:512], s1[:, 0:512],
                                 clbias[:, h * 512:(h + 1) * 512])
            return s1

        b1pre = gemm1_prologue()
        b2r_init()
        for r in range(N_ROUNDS):
            ag_outs = []
            for h in range(2):
                ag_out = dram.tile([4096, DIM], FP8, tag=f"ag_out{h}",
                                   addr_space="Shared", name=f"ag_out{h}_{r}")
                collective("AllGather", mybir.AluOpType.bypass,
                           ag_ins[h], ag_out)
                ag_outs.append(ag_out)
            lpre_sb = load_lpre(ag_outs)
            lct_ps = gemm1(lpre_sb, N_WARM_G1, b1pre)
            if r + 1 < N_ROUNDS:
                b1pre = gemm1_prologue()
            b2s = [b2_fetch(i) for i in range(min(N_B2_STREAM, N_B2_PRE))]

            c_lstm(lct_ps)
            cpre_kt = sb.tile([DIM, KC * DIM], FP8, tag="cpre_kt", bufs=1,
                              name="cpre_kt")
            c_mlp_half(0, cpre_kt)
            c_mlp_half(1, cpre_kt)

            lh_flip = sb.tile([DIM, LL], BF16, tag="lh_flip", bufs=1,
                              name="lh_flip")
            nc.scalar.activation(lh_flip[:], lht[:], AF.Identity)

            rs_bufs = [dram.tile([N_CORES * DIM, 512], FP8, tag=f"rs_in{h}",
                                 name=f"rs_in{h}_{r}") for h in range(2)]
            gemm2_group(cpre_kt, 0, rs_bufs, r, b2s)
            gemm2_group(cpre_kt, 1, rs_bufs, r, b2s)
            ro0 = dram.tile([N_CORES * DIM, 512], FP8, tag="rs_out0",
                            name=f"rs_out0_{r}")
            collective("AllToAll", mybir.AluOpType.bypass, rs_bufs[0], ro0)
            gemm2_group(cpre_kt, 2, rs_bufs, r, b2s)
            gemm2_group(cpre_kt, 3, rs_bufs, r, b2s)

            ro1 = dram.tile([N_CORES * DIM, 512], FP8, tag="rs_out1",
                            name=f"rs_out1_{r}")
            collective("AllToAll", mybir.AluOpType.bypass, rs_bufs[1], ro1)

            last = r + 1 == N_ROUNDS   # final round: no AG follows, skip
            ag_ins = [None if last else
                      dram.tile([512, DIM], FP8, tag=f"ag_in{h}",
                                name=f"ag_in{h}_{r}") for h in range(2)]
            sb0 = a2a_load(0, ro0, r)
            sb1 = a2a_load(1, ro1, r)
            clt0 = a2a_sum(0, sb0, r)
            l_half(0, clt0, lh_flip, r, N_WARM1, ag_ins[0])

            clt1 = a2a_sum(1, sb1, r)
            l_half(1, clt1, lh_flip, r, N_WARM2, ag_ins[1])

        # ---- vote MLP on final Lh -> [1, 1024] f32
        vt0 = mlp_chunk(lht, "lvote", slice(0, 512), 512, out_dt=F32,
                        tagsfx="_0")
        vt1 = mlp_chunk(lht, "lvote", slice(512, 1024), 512, out_dt=F32,
                        tagsfx="_1")
        nc.sync.dma_start(vote_out.ap()[:, 0:512], vt0[:])
        nc.sync.dma_start(vote_out.ap()[:, 512:1024], vt1[:])

    nc.compile()
    _CACHE["nc"] = nc
    return nc


def _perm_rows(lits):
    """Map global lit index -> permuted row (core-major, 1024 rows/core)."""
    lits = np.asarray(lits)
    neg = lits >= N_VARS
    v = np.where(neg, lits - N_VARS, lits)
    core = v // VPC
    r = v % VPC
    return core * LL + np.where(neg, VPAD + r, r)


def _b1_row_order():
    """B1 rows: [half h, core c, r] -> permuted row c*1024 + h*512 + r."""
    order = np.empty(LPAD, np.int64)
    n = 0
    for h in range(2):
        for c in range(N_CORES):
            order[n:n + 512] = c * LL + h * 512 + np.arange(512)
            n += 512
    return order


def host_prep(inp):
    f32 = np.float32
    idx = inp["L_unpack_indices"].astype(np.int64)
    rows = _perm_rows(idx[:, 0])
    M = np.zeros((LPAD, CPAD), np.float32)
    np.add.at(M, (rows, idx[:, 1]), 1.0)

    # degree vectors for the folded layer-3 biases
    deg_c = M.sum(axis=0)                        # [CPAD] clause degrees
    deg_l = M.sum(axis=1)                        # [LPAD] permuted lit degrees
    lmsg_b3 = inp["Lmsg_b3"].astype(f32)
    cmsg_b3 = inp["Cmsg_b3"].astype(f32)

    row_order = _b1_row_order()
    b1s, b2s, lcbs, clbs = [], [], [], []
    for i in range(N_CORES):
        blk = M[:, i * CC:(i + 1) * CC]          # [8192, 2048] permuted rows
        b1o = blk[row_order]                      # AG-concat row order
        # pack 4 k-tiles per DMA group: [16, 128, 4*2048]
        b1p = b1o.reshape(16, 4, DIM, CC).transpose(0, 2, 1, 3) \
                 .reshape(16, DIM, 4 * CC)
        b1s.append(np.ascontiguousarray(b1p).astype(nf8))
        bT = blk.T                                # [2048 clauses, 8192 lits]
        grp = []
        for g in range(4):
            cols = np.concatenate([np.arange(j * 512, (j + 1) * 512)
                                   for j in J_SETS[g]])
            gb = bT[:, cols]                      # [2048, 2048]
            gp = gb.reshape(4, 4, DIM, 2048).transpose(0, 2, 1, 3) \
                   .reshape(4, DIM, 4 * 2048)
            grp.append(gp)
        b2s.append(np.ascontiguousarray(np.stack(grp)).astype(nf8))
        lcbs.append(np.outer(lmsg_b3, deg_c[i * CC:(i + 1) * CC]))
        clbs.append(np.outer(cmsg_b3, deg_l[i * LL:(i + 1) * LL]))

    def bf(x):
        return np.ascontiguousarray(x).astype(nbf)

    l0 = (inp["L_init_w"][:, 0] + inp["L_init_b"]).astype(f32)
    c0 = (inp["C_init_w"][:, 0] + inp["C_init_b"]).astype(f32)
    common = {
        "lh0t": bf(np.repeat(l0[:, None], LL, axis=1)),
        "ch0t": bf(np.repeat(c0[:, None], CC, axis=1)),
        "id128": bf(np.eye(DIM, dtype=f32)),
        "cu_wiht": bf(inp["Cu_wih"].T), "cu_whht": bf(inp["Cu_whh"].T),
        "lu_wiht_cl": bf(inp["Lu_wih"].T[:DIM]),
        "lu_wiht_fl": bf(inp["Lu_wih"].T[DIM:]),
        "lu_whht": bf(inp["Lu_whh"].T),
        "cu_bias": (inp["Cu_bih"] + inp["Cu_bhh"]).astype(f32).reshape(4, DIM),
        "lu_bias": (inp["Lu_bih"] + inp["Lu_bhh"]).astype(f32).reshape(4, DIM),
    }
    for p, P in (("lmsg", "Lmsg"), ("cmsg", "Cmsg"), ("lvote", "Lvote")):
        for i in (1, 2, 3):
            common[f"{p}_w{i}t"] = bf(inp[f"{P}_w{i}"].T)
            bshape = (1, 1) if (p == "lvote" and i == 3) else (DIM, 1)
            common[f"{p}_b{i}"] = inp[f"{P}_b{i}"].astype(f32).reshape(bshape)
    return [dict(common, b1=b1s[i], b2=b2s[i], lcbias=bf(lcbs[i]),
                 clbias=bf(clbs[i])) for i in range(N_CORES)]


def kernel(**inputs):
    inp = {k: np.asarray(v) for k, v in inputs.items()}
    in_maps = host_prep(inp)
    nc = _build()
    res = bass_utils.run_bass_kernel_spmd(nc, in_maps,
                                          core_ids=list(range(N_CORES)))
    probs = np.zeros(N_CORES, np.float32)
    for i in range(N_CORES):
        v = res.results[i]["vote"][0]            # [1024]
        s = v[:VPC].astype(np.float64).sum() + \
            v[VPAD:VPAD + VPC].astype(np.float64).sum()
        probs[i] = np.float32(s / (2 * VPC))
    return probs

